# revision 1
# baseline (speedup 1.0000x reference)
"""Trainium2 Bass kernel for nn_DDPM (fused dynamic per-pixel conv DDPM block).

Contract: kernel(**inputs) takes FULL inputs (x, y, gen_w, gen_b, fuse_w,
fuse_b) as numpy arrays and returns the FULL [4, 64, 128, 128] fp32 output.

Sharding: 8 cores = 4 images x 2 H-halves, pure data parallel. Halos are
materialized host-side (each core receives its slice plus halo rows), so no
collectives are needed.

Per-core dataflow (partition layout [c + 64*g], g = row-group 0/1, each group
covers 34 "cat rows" = 32 output rows + 1 halo row each side, groups overlap
by 2 rows):
  1. gen matmul (PE, fp32r): k-planes for the 27 (branch, tap) combos,
     col-tiled so group 0 lands in psum[0:64] and group 1 in psum[64:128].
  2. ACT evacuates psum -> SBUF fp16 with gen_b bias folded in.
  3. DVE (fp16, 2x mode): 27 products + 24 accumulating adds -> 3 branch
     tiles; tiny per-partition mask multiplies zero the out-of-image rows.
  4. DMA repack (SBUF crossbar) into cat tiles [x|b1], [b2|b3] per group.
  5. fuse conv (PE, fp16): 9 spatial taps x 2 K-tiles, col-tiled by group;
     ACT evacuates with fuse_b bias; DMA out.
"""

import sys

for _p in ("/opt/trn_rl_repo", "/root/.axon_site/_ro/trn_rl_repo"):
    if _p not in sys.path:
        sys.path.insert(0, _p)

import numpy as np

# ---------------------------------------------------------------- constants
N, C, H, W = 4, 64, 128, 128
KS = 3
DIL = (1, 3, 5)
NCORES = 8
RG = 34       # cat rows per row-group
XH = 44       # x rows per group slice (RG + 2*5)
WP = 138      # padded x width (W + 2*5)
CATW = 132    # cat width: w = -2..129, w=0 at column 2
OUTR = 32     # output rows per group

F16 = np.float16
F32 = np.float32


# ------------------------------------------------------------- host packing
def _prep_cores(x, y):
    """Per-core input slices. Returns dict name -> [NCORES, ...] arrays."""
    xe = np.zeros((NCORES, 128, XH, WP), F16)
    xo = np.zeros((NCORES, 128, XH, WP), F16)
    ys = np.zeros((NCORES, 128, 66, W), F16)
    m0 = np.ones((NCORES, 128, 1), F32)
    m33 = np.ones((NCORES, 128, 1), F32)
    for core in range(NCORES):
        n, hh = core // 2, core % 2
        h0 = 64 * hh
        for g in range(2):
            r0 = h0 + 32 * g - 6
            lo, hi = max(0, -r0), min(XH, H - r0)
            if hi > lo:
                xe[core, 64 * g:64 * g + 64, lo:hi, 5:5 + W] = x[n, :, r0 + lo:r0 + hi, :]
        xo[core, :, :, :-1] = xe[core, :, :, 1:]
        r0 = h0 - 1
        lo, hi = max(0, -r0), min(66, H - r0)
        ys[core, 0:64, lo:hi, :] = y[n, :, r0 + lo:r0 + hi, :]
        ys[core, 64:128] = ys[core, 0:64]
        if h0 == 0:
            m0[core, 0:64] = 0.0
        if h0 + 64 == H:
            m33[core, 64:128] = 0.0
    return {"xe": xe, "xo": xo, "ys": ys, "m0": m0, "m33": m33}


def _prep_weights(gen_w, gen_b, fuse_w, fuse_b):
    """Weight rearrangement (shared across cores)."""
    # gen lhsT [cy, 64*j + cout], plane j = bi*9 + t; original row o = bi*576 + c*9 + t
    gw = np.empty((128, 27 * 64), F16)
    gb = np.empty((128, 27), F32)
    for bi in range(3):
        for t in range(9):
            j = bi * 9 + t
            o = bi * 576 + np.arange(64) * 9 + t
            gw[0:64, 64 * j:64 * j + 64] = gen_w[o, :].T
            gw[64:128, 64 * j:64 * j + 64] = gen_w[o, :].T
            gb[0:64, j] = gen_b[o]
            gb[64:128, j] = gen_b[o]
    # fuse lhsT [kc, (kt*9+s)*64 + o]
    fwT = np.empty((128, 18 * 64), F16)
    for kt in range(2):
        for s in range(9):
            sh, sw = s // 3, s % 3
            base = 64 if kt == 0 else 128
            # kt=0: channels [x(0:64) | b1(64:128)] -> cat ch kc (kc<64: x ch kc; else 64+kc-64)
            # kt=1: channels [b2 | b3] -> 128+kc / 192+(kc-64)
            for half in range(2):
                ch0 = (0 if half == 0 else 64) if kt == 0 else (128 if half == 0 else 192)
                blk = fuse_w[:, ch0:ch0 + 64, sh, sw].T.astype(F16)  # [kc_local, o]
                fwT[64 * half:64 * half + 64, (kt * 9 + s) * 64:(kt * 9 + s) * 64 + 64] = blk
    fb = np.empty((128, 1), F32)
    fb[0:64, 0] = fuse_b
    fb[64:128, 0] = fuse_b
    return {"gw": gw, "gb": gb, "fwT": fwT, "fb": fb}


# ------------------------------------------------------------- bass builder
def _build_nc(reps=1):
    import os as _os
    _skip = set(_os.environ.get("KERNEL_SKIP", "").split(","))
    import concourse.bass as bass
    import concourse.tile as tile
    import concourse.mybir as mybir
    from concourse import bacc

    dt = mybir.dt
    MULT = mybir.AluOpType.mult
    ADD = mybir.AluOpType.add
    IDENT = mybir.ActivationFunctionType.Identity

    nc = bacc.Bacc("TRN2", target_bir_lowering=False, debug=False, num_devices=NCORES)

    d_xe = nc.dram_tensor("xe", [128, XH, WP], dt.float16, kind="ExternalInput")
    d_xo = nc.dram_tensor("xo", [128, XH, WP], dt.float16, kind="ExternalInput")
    d_ys = nc.dram_tensor("ys", [128, 66, W], dt.float16, kind="ExternalInput")
    d_gw = nc.dram_tensor("gw", [128, 27 * 64], dt.float16, kind="ExternalInput")
    d_gb = nc.dram_tensor("gb", [128, 27], dt.float32, kind="ExternalInput")
    d_fwT = nc.dram_tensor("fwT", [128, 18 * 64], dt.float16, kind="ExternalInput")
    d_fb = nc.dram_tensor("fb", [128, 1], dt.float32, kind="ExternalInput")
    d_m0 = nc.dram_tensor("m0", [128, 1], dt.float32, kind="ExternalInput")
    d_m33 = nc.dram_tensor("m33", [128, 1], dt.float32, kind="ExternalInput")
    d_out = nc.dram_tensor("out", [128, OUTR, W], dt.float32, kind="ExternalOutput")

    import contextlib

    with tile.TileContext(nc) as tc:
        with (
            tc.tile_pool(name="const", bufs=1) as constp,
            tc.tile_pool(name="xpool", bufs=1) as xpool,
            tc.tile_pool(name="kpool", bufs=4) as kpool,
            tc.tile_pool(name="prodpool", bufs=2) as prodpool,
            tc.tile_pool(name="bpool", bufs=1) as bpool,
            tc.tile_pool(name="catpool", bufs=1) as catpool,
            tc.tile_pool(name="outpool", bufs=2) as outpool,
            tc.tile_pool(name="genps", bufs=6, space="PSUM") as genps,
            tc.tile_pool(name="fuseps", bufs=1, space="PSUM") as fuseps,
            tc.tile_pool(name="fusepsB", bufs=1, space="PSUM") as fusepsB,
        ):
          with (tc.For_i(0, reps, 1) if reps > 1 else contextlib.nullcontext()):
            # ---- input loads
            t_xe = xpool.tile([128, XH, WP], dt.float16, tag="xe")
            nc.gpsimd.dma_start(t_xe[:], d_xe[:])
            t_xo = xpool.tile([128, XH, WP], dt.float16, tag="xo")
            nc.gpsimd.dma_start(t_xo[:], d_xo[:])
            t_ys = xpool.tile([128, 66, W], dt.float16, tag="ys")
            nc.gpsimd.dma_start(t_ys[:], d_ys[:])
            t_gw = constp.tile([128, 27 * 64], dt.float16, tag="gw")
            nc.gpsimd.dma_start(t_gw[:], d_gw[:])
            t_gb = constp.tile([128, 27], dt.float32, tag="gb")
            nc.gpsimd.dma_start(t_gb[:], d_gb[:])
            t_fwT = constp.tile([128, 18 * 64], dt.float16, tag="fwT")
            nc.gpsimd.dma_start(t_fwT[:], d_fwT[:])
            t_fb = constp.tile([128, 1], dt.float32, tag="fb")
            nc.gpsimd.dma_start(t_fb[:], d_fb[:])
            t_m0 = constp.tile([128, 1], dt.float32, tag="m0")
            nc.gpsimd.dma_start(t_m0[:], d_m0[:])
            t_m33 = constp.tile([128, 1], dt.float32, tag="m33")
            nc.gpsimd.dma_start(t_m33[:], d_m33[:])

            # ---- branch accumulator tiles (persist across the plane loop)
            t_b = [bpool.tile([128, RG, CATW], dt.float16, tag=f"b{bi}", name=f"b{bi}") for bi in range(3)]
            # zero the W-pad columns (cols 0,1,130,131) once; products cover the rest
            for bi in range(3):
                nc.vector.memset(t_b[bi][:, :, 0:2], 0.0)
                nc.vector.memset(t_b[bi][:, :, 130:132], 0.0)

            # ---- two row-ranges; gen is emitted in plane PAIRS on alternating
            # PE row-groups (so LDWEIGHTS of one overlaps MATMUL of the other);
            # range-0 fuse chunks are interleaved into range-1's plane loop so
            # the PE stream never serializes a whole phase.
            RANGES = [(0, 18), (18, 34)]
            CATR0 = [0, 14]
            CATN = [18, 20]
            FCH = [range(0, 4), range(4, 8)]

            t_cat = [[[catpool.tile([128, CATN[ri], CATW], dt.float16,
                                    tag=f"cat{ri}{kt}{g}", name=f"cat{ri}{kt}{g}")
                       for g in range(2)] for kt in range(2)] for ri in range(2)]

            state = {"t_out": None}

            def emit_gen_pair(pair, R0, R1):
                kpls = {}
                for j in pair:
                    kpls[j] = kpool.tile([128, 18, W], dt.float16, tag="kpl", name="kpl")
                c0 = R0
                while c0 < R1:
                    c1 = min(c0 + 4, R1)
                    npx = (c1 - c0) * W
                    pss = {}
                    for j in pair:
                        pss[j] = genps.tile([128, 512], dt.float32, tag="genps", name="genps")
                    for s0 in range(c0, c1, 4):
                        s1 = min(s0 + 4, c1)
                        for g in range(2):
                            for j in pair:
                                rg = 64 * (j % 2)
                                nc.tensor.matmul(
                                    pss[j][64 * g:64 * g + 64, (s0 - c0) * W:(s1 - c0) * W],
                                    t_gw[rg:rg + 64, 64 * j:64 * j + 64],
                                    t_ys[rg:rg + 64, 32 * g + s0:32 * g + s1, :],
                                    start=True, stop=True,
                                    tile_position=(rg, 64 * g),
                                )
                    for j in pair:
                        nc.scalar.activation(
                            kpls[j][:, c0 - R0:c1 - R0, :],
                            pss[j][:, 0:npx].rearrange("p (r w) -> p r w", w=W),
                            IDENT, bias=t_gb[:, j:j + 1], scale=1.0,
                        )
                    c0 = c1
                return kpls

            def emit_products(j, kpl, R0, R1):
                if "prod" in _skip:
                    nc.vector.tensor_copy(t_b[0][:, 0:1, 0:1], kpl[:, 0:1, 0:1])
                    return
                bi, t = j // 9, j % 9
                d = DIL[bi]
                dh, dw = t // 3 - 1, t % 3 - 1
                nrows = R1 - R0
                row_ofs = 5 + dh * d
                col_ofs = 5 + dw * d
                if col_ofs % 2 == 0:
                    xv = t_xe[:, row_ofs + R0:row_ofs + R1, col_ofs:col_ofs + W]
                else:
                    xv = t_xo[:, row_ofs + R0:row_ofs + R1, col_ofs - 1:col_ofs - 1 + W]
                bint = t_b[bi][:, R0:R1, 2:2 + W]
                if t == 0:
                    nc.vector.tensor_tensor(bint, kpl[:, 0:nrows, :], xv, MULT)
                else:
                    prod = prodpool.tile([128, 18, W], dt.float16, tag="prod", name="prod")
                    nc.vector.tensor_tensor(prod[:, 0:nrows, :], kpl[:, 0:nrows, :], xv, MULT)
                    nc.vector.tensor_tensor(bint, bint, prod[:, 0:nrows, :], ADD)

            def emit_fuse_chunk(ri, ch):
                if "fuse" in _skip:
                    return
                cr0 = CATR0[ri]
                if ch % 2 == 0:
                    state["t_out"] = outpool.tile([128, 8, W], dt.float32, tag="out", name="outt")
                t_out = state["t_out"]
                psA = fuseps.tile([128, 512], dt.float32, tag="fuseps", name="psA")
                psB = fusepsB.tile([128, 512], dt.float32, tag="fusepsB", name="psB")
                ops = [(kt, s) for kt in range(2) for s in range(9)]
                for g in range(2):
                    for i, (kt, s) in enumerate(ops):
                        sh, sw = s // 3 - 1, s % 3 - 1
                        rr = 1 + 4 * ch + sh - cr0
                        blk = slice((kt * 9 + s) * 64, (kt * 9 + s) * 64 + 64)
                        cat_g = t_cat[ri][kt][g]
                        nc.tensor.matmul(
                            psA[64 * g:64 * g + 64, :], t_fwT[0:64, blk],
                            cat_g[0:64, rr:rr + 4, 2 + sw:2 + sw + W],
                            start=(i == 0), stop=(i == len(ops) - 1),
                            tile_position=(0, 64 * g),
                        )
                        nc.tensor.matmul(
                            psB[64 * g:64 * g + 64, :], t_fwT[64:128, blk],
                            cat_g[64:128, rr:rr + 4, 2 + sw:2 + sw + W],
                            start=(i == 0), stop=(i == len(ops) - 1),
                            tile_position=(64, 64 * g),
                        )
                oview = t_out[:, 4 * (ch % 2):4 * (ch % 2) + 4, :]
                nc.scalar.activation(
                    oview, psA[:].rearrange("p (r w) -> p r w", w=W),
                    IDENT, bias=t_fb[:, 0:1], scale=1.0,
                )
                nc.vector.scalar_tensor_tensor(
                    oview.rearrange("p r w -> p (r w)"), oview.rearrange("p r w -> p (r w)"),
                    0.0, psB[:], mybir.AluOpType.add, mybir.AluOpType.add,
                )
                if ch % 2 == 1:
                    nc.gpsimd.dma_start(d_out[:, 4 * ch - 4:4 * ch + 4, :], t_out[:])

            def emit_mask_and_cat(ri):
                mrow = 0 if ri == 0 else 33
                mt = t_m0 if ri == 0 else t_m33
                for bi in range(3):
                    nc.vector.tensor_scalar_mul(t_b[bi][:, mrow, :], t_b[bi][:, mrow, :], mt[:, 0:1])
                cr0, crn = CATR0[ri], CATN[ri]
                for g in range(2):
                    sl = slice(64 * g, 64 * g + 64)
                    nc.gpsimd.dma_start(t_cat[ri][0][g][0:64, :, :],
                                        t_xe[sl, 5 + cr0:5 + cr0 + crn, 3:3 + CATW])
                    nc.gpsimd.dma_start(t_cat[ri][0][g][64:128, :, :], t_b[0][sl, cr0:cr0 + crn, :])
                    nc.gpsimd.dma_start(t_cat[ri][1][g][0:64, :, :], t_b[1][sl, cr0:cr0 + crn, :])
                    nc.gpsimd.dma_start(t_cat[ri][1][g][64:128, :, :], t_b[2][sl, cr0:cr0 + crn, :])

            pairs = [[jp] if jp == 26 else [jp, jp + 1] for jp in range(0, 27, 2)]

            # range 0: gen + products
            for pair in pairs:
                kpls = emit_gen_pair(pair, 0, 18)
                for j in pair:
                    emit_products(j, kpls[j], 0, 18)
            emit_mask_and_cat(0)

            # range 1 with range-0 fuse chunks interleaved (PE program order!)
            fuse0 = list(FCH[0])
            for pi, pair in enumerate(pairs):
                kpls = emit_gen_pair(pair, 18, 34)
                for j in pair:
                    emit_products(j, kpls[j], 18, 34)
                if pi in (2, 5, 8, 11):
                    emit_fuse_chunk(0, fuse0.pop(0))
            while fuse0:
                emit_fuse_chunk(0, fuse0.pop(0))
            emit_mask_and_cat(1)
            for ch in FCH[1]:
                emit_fuse_chunk(1, ch)

    nc.compile()
    return nc


# ----------------------------------------------------------------- runner
_CACHE = {}


def _get_runner(reps=1):
    """Build (once) a persistent jitted 8-core runner: fn(core_inputs) -> [out]*8."""
    key = ("runner", reps)
    if key in _CACHE:
        return _CACHE[key]

    import jax
    import numpy as _np
    from jax.sharding import Mesh, PartitionSpec
    from jax.experimental.shard_map import shard_map
    from concourse import mybir
    from concourse.bass2jax import install_neuronx_cc_hook, _bass_exec_p, partition_id_tensor

    nc = _build_nc(reps)
    install_neuronx_cc_hook()

    partition_name = nc.partition_id_tensor.name if nc.partition_id_tensor else None
    in_names, out_names, out_avals = [], [], []
    for alloc in nc.m.functions[0].allocations:
        if not isinstance(alloc, mybir.MemoryLocationSet):
            continue
        name = alloc.memorylocations[0].name
        if alloc.kind == "ExternalInput":
            if name != partition_name:
                in_names.append(name)
        elif alloc.kind == "ExternalOutput":
            out_names.append(name)
            out_avals.append(
                jax.core.ShapedArray(tuple(alloc.tensor_shape), mybir.dt.np(alloc.dtype))
            )
    n_params = len(in_names)
    n_outs = len(out_names)
    all_names = in_names + out_names
    if partition_name is not None:
        all_names = all_names + [partition_name]

    def _body(*args):
        operands = list(args)
        if partition_name is not None:
            operands.append(partition_id_tensor())
        outs = _bass_exec_p.bind(
            *operands,
            out_avals=tuple(out_avals),
            in_names=tuple(all_names),
            out_names=tuple(out_names),
            lowering_input_output_aliases=(),
            sim_require_finite=True,
            sim_require_nnan=True,
            nc=nc,
        )
        return tuple(outs)

    devices = jax.devices()[:NCORES]
    mesh = Mesh(_np.asarray(devices), ("core",))
    in_specs = (PartitionSpec("core"),) * (n_params + n_outs)
    out_specs = (PartitionSpec("core"),) * n_outs
    sharded = jax.jit(
        shard_map(_body, mesh=mesh, in_specs=in_specs, out_specs=out_specs, check_rep=False),
        keep_unused=True,
    )

    zero_shapes = [(NCORES * a.shape[0], *a.shape[1:]) for a in out_avals]
    zero_dtypes = [a.dtype for a in out_avals]

    def run(concat_inputs):
        """concat_inputs: list (len n_params, order in_names) of [NCORES*dim0, ...]."""
        zeros = [_np.zeros(s, d) for s, d in zip(zero_shapes, zero_dtypes)]
        out_arrs = sharded(*concat_inputs, *zeros)
        return [
            _np.asarray(out_arrs[i]).reshape(NCORES, *out_avals[i].shape)
            for i in range(n_outs)
        ]

    _CACHE[key] = (run, in_names, out_names)
    _CACHE[("raw", reps)] = dict(sharded=sharded, zero_shapes=zero_shapes, zero_dtypes=zero_dtypes,
                                 n_params=n_params, n_outs=n_outs, out_avals=out_avals, mesh=mesh)
    return _CACHE[key]


def make_concat_inputs(x, y, gen_w, gen_b, fuse_w, fuse_b, in_names):
    per_core = _prep_cores(np.asarray(x, F32), np.asarray(y, F32))
    wts = _prep_weights(
        np.asarray(gen_w, F32), np.asarray(gen_b, F32),
        np.asarray(fuse_w, F32), np.asarray(fuse_b, F32),
    )
    cat = []
    for name in in_names:
        if name in per_core:
            a = per_core[name]
            cat.append(a.reshape(NCORES * a.shape[1], *a.shape[2:]))
        else:
            w = wts[name]
            cat.append(np.concatenate([w] * NCORES, axis=0))
    return cat


def unpack_output(outs):
    """outs: [NCORES, 128, OUTR, W] fp32 -> [N, C, H, W]."""
    res = np.empty((N, C, H, W), F32)
    for core in range(NCORES):
        n, hh = core // 2, core % 2
        h0 = 64 * hh
        for g in range(2):
            res[n, :, h0 + 32 * g:h0 + 32 * g + 32, :] = outs[core, 64 * g:64 * g + 64]
    return res


def kernel(x, y, gen_w, gen_b, fuse_w, fuse_b):
    run, in_names, out_names = _get_runner()
    cat = make_concat_inputs(x, y, gen_w, gen_b, fuse_w, fuse_b, in_names)
    outs = run(cat)
    return unpack_output(outs[out_names.index("out")])



# revision 6
# speedup vs baseline: 6139.2677x; 6139.2677x over previous
"""Trainium2 Bass kernel for nn_DDPM (fused dynamic per-pixel conv DDPM block).

Contract: kernel(**inputs) takes FULL inputs (x, y, gen_w, gen_b, fuse_w,
fuse_b) as numpy arrays and returns the FULL [4, 64, 128, 128] fp32 output.

Sharding: 8 cores = 4 images x 2 H-halves, pure data parallel. Halos are
materialized host-side (each core receives its slice plus halo rows), so no
collectives are needed.

Per-core dataflow (partition layout [c + 64*g], g = row-group 0/1, each group
covers 34 "cat rows" = 32 output rows + 1 halo row each side, groups overlap
by 2 rows):
  1. gen matmul (PE, fp16): block-diagonal weights [gw 0; 0 gw] compute each
     k-plane for BOTH row-groups in one matmul (ysg partition halves hold the
     two groups' row windows); 8-row (2-psum-bank) granules per plane.
  2. psum evacuation with gen_b bias folded in -> SBUF fp16 kpl planes;
     split between ACT (activation w/ bias) and Pool (tensor_scalar_add)
     to balance engine load.
  3. DVE (fp16, 2x mode): 27 products + 24 accumulating adds -> 3 branch
     tiles; tiny per-partition mask multiplies zero the out-of-image rows.
  4. DMA repack (SBUF crossbar, issued on SP/HWDGE) into cat tiles
     [x|b1], [b2|b3] per group.
  5. fuse conv (PE, fp16): 9 spatial taps x 2 K-tiles with the full
     128-partition contraction per matmul, both groups accumulating into one
     psum bank; single ACT evacuates with fuse_b bias; DMA out.
"""

import sys

for _p in ("/opt/trn_rl_repo", "/root/.axon_site/_ro/trn_rl_repo"):
    if _p not in sys.path:
        sys.path.insert(0, _p)

import numpy as np

# ---------------------------------------------------------------- constants
N, C, H, W = 4, 64, 128, 128
KS = 3
DIL = (1, 3, 5)
NCORES = 8
RG = 34       # cat rows per row-group
XH = 44       # x rows per group slice (RG + 2*5)
WP = 138      # padded x width (W + 2*5)
CATW = 132    # cat width: w = -2..129, w=0 at column 2
OUTR = 32     # output rows per group

F16 = np.float16
F32 = np.float32


# ------------------------------------------------------------- host packing
def _prep_cores(x, y):
    """Per-core input slices. Returns dict name -> [NCORES, ...] arrays."""
    xe = np.zeros((NCORES, 128, XH, WP), F16)
    xo = np.zeros((NCORES, 128, XH, WP), F16)
    ys = np.zeros((NCORES, 128, RG, W), F16)
    m0 = np.ones((NCORES, 128, 1), F32)
    m33 = np.ones((NCORES, 128, 1), F32)
    for core in range(NCORES):
        n, hh = core // 2, core % 2
        h0 = 64 * hh
        for g in range(2):
            r0 = h0 + 32 * g - 6
            lo, hi = max(0, -r0), min(XH, H - r0)
            if hi > lo:
                xe[core, 64 * g:64 * g + 64, lo:hi, 5:5 + W] = x[n, :, r0 + lo:r0 + hi, :]
            # y rows for group g: image rows h0 + 32*g - 1 .. + 32
            yr0 = h0 + 32 * g - 1
            ylo, yhi = max(0, -yr0), min(RG, H - yr0)
            if yhi > ylo:
                ys[core, 64 * g:64 * g + 64, ylo:yhi, :] = y[n, :, yr0 + ylo:yr0 + yhi, :]
        xo[core, :, :, :-1] = xe[core, :, :, 1:]
        if h0 == 0:
            m0[core, 0:64] = 0.0
        if h0 + 64 == H:
            m33[core, 64:128] = 0.0
    return {"xe": xe, "xo": xo, "ys": ys, "m0": m0, "m33": m33}


def _prep_weights(gen_w, gen_b, fuse_w, fuse_b):
    """Weight rearrangement (shared across cores)."""
    # gen lhsT: block-diagonal [gw_j 0; 0 gw_j] per plane j = bi*9 + t so one
    # matmul computes plane j for both row-groups; original row o = bi*576 + c*9 + t
    gw = np.zeros((128, 27 * 128), F16)
    gb = np.empty((128, 27), F32)
    for bi in range(3):
        for t in range(9):
            j = bi * 9 + t
            o = bi * 576 + np.arange(64) * 9 + t
            blkT = gen_w[o, :].T  # [cy, cout]
            gw[0:64, 128 * j:128 * j + 64] = blkT
            gw[64:128, 128 * j + 64:128 * j + 128] = blkT
            gb[0:64, j] = gen_b[o]
            gb[64:128, j] = gen_b[o]
    # fuse lhsT [kc, (kt*9+s)*64 + o]
    fwT = np.empty((128, 18 * 64), F16)
    for kt in range(2):
        for s in range(9):
            sh, sw = s // 3, s % 3
            # kt=0: channels [x(0:64) | b1(64:128)] ; kt=1: [b2 | b3]
            for half in range(2):
                ch0 = (0 if half == 0 else 64) if kt == 0 else (128 if half == 0 else 192)
                blk = fuse_w[:, ch0:ch0 + 64, sh, sw].T.astype(F16)  # [kc_local, o]
                fwT[64 * half:64 * half + 64, (kt * 9 + s) * 64:(kt * 9 + s) * 64 + 64] = blk
    fb = np.empty((128, 1), F32)
    fb[0:64, 0] = fuse_b
    fb[64:128, 0] = fuse_b
    return {"gw": gw, "gb": gb, "fwT": fwT, "fb": fb}


# ------------------------------------------------------------- bass builder
def _build_nc(reps=1, pool_evac=0):
    """pool_evac: every pool_evac-th gen psum granule is evacuated on the Pool
    engine (tensor_scalar_add) instead of ACT, to balance engine load.
    0 disables Pool evacuation."""
    import os as _os
    _skip = set(_os.environ.get("KERNEL_SKIP", "").split(","))
    import concourse.bass as bass
    import concourse.tile as tile
    import concourse.mybir as mybir
    from concourse import bacc

    dt = mybir.dt
    MULT = mybir.AluOpType.mult
    ADD = mybir.AluOpType.add
    IDENT = mybir.ActivationFunctionType.Identity

    nc = bacc.Bacc("TRN2", target_bir_lowering=False, debug=False, num_devices=NCORES)

    d_xe = nc.dram_tensor("xe", [128, XH, WP], dt.float16, kind="ExternalInput")
    d_xo = nc.dram_tensor("xo", [128, XH, WP], dt.float16, kind="ExternalInput")
    d_ys = nc.dram_tensor("ys", [128, RG, W], dt.float16, kind="ExternalInput")
    d_gw = nc.dram_tensor("gw", [128, 27 * 128], dt.float16, kind="ExternalInput")
    d_gb = nc.dram_tensor("gb", [128, 27], dt.float32, kind="ExternalInput")
    d_fwT = nc.dram_tensor("fwT", [128, 18 * 64], dt.float16, kind="ExternalInput")
    d_fb = nc.dram_tensor("fb", [128, 1], dt.float32, kind="ExternalInput")
    d_m0 = nc.dram_tensor("m0", [128, 1], dt.float32, kind="ExternalInput")
    d_m33 = nc.dram_tensor("m33", [128, 1], dt.float32, kind="ExternalInput")
    d_out = nc.dram_tensor("out", [128, OUTR, W], dt.float32, kind="ExternalOutput")

    import contextlib

    with tile.TileContext(nc) as tc:
        with (
            tc.tile_pool(name="const", bufs=1) as constp,
            tc.tile_pool(name="xpool", bufs=1) as xpool,
            tc.tile_pool(name="kpool", bufs=4) as kpool,
            tc.tile_pool(name="prodpool", bufs=2) as prodpool,
            tc.tile_pool(name="bpool", bufs=1) as bpool,
            tc.tile_pool(name="catpool", bufs=1) as catpool,
            tc.tile_pool(name="outpool", bufs=2) as outpool,
            tc.tile_pool(name="genps", bufs=3, space="PSUM") as genps,
            tc.tile_pool(name="fuseps", bufs=2, space="PSUM") as fuseps,
        ):
          with (tc.For_i(0, reps, 1) if reps > 1 else contextlib.nullcontext()):
            # ---- input loads (SP queue: HWDGE, no engine time)
            t_xe = xpool.tile([128, XH, WP], dt.float16, tag="xe")
            nc.scalar.dma_start(t_xe[:], d_xe[:])
            t_xo = xpool.tile([128, XH, WP], dt.float16, tag="xo")
            nc.scalar.dma_start(t_xo[:], d_xo[:])
            t_ys = xpool.tile([128, RG, W], dt.float16, tag="ys")
            nc.scalar.dma_start(t_ys[:], d_ys[:])
            t_gw = constp.tile([128, 27 * 128], dt.float16, tag="gw")
            nc.scalar.dma_start(t_gw[:], d_gw[:])
            t_gb = constp.tile([128, 27], dt.float32, tag="gb")
            nc.scalar.dma_start(t_gb[:], d_gb[:])
            t_fwT = constp.tile([128, 18 * 64], dt.float16, tag="fwT")
            nc.scalar.dma_start(t_fwT[:], d_fwT[:])
            t_fb = constp.tile([128, 1], dt.float32, tag="fb")
            nc.scalar.dma_start(t_fb[:], d_fb[:])
            t_m0 = constp.tile([128, 1], dt.float32, tag="m0")
            nc.scalar.dma_start(t_m0[:], d_m0[:])
            t_m33 = constp.tile([128, 1], dt.float32, tag="m33")
            nc.scalar.dma_start(t_m33[:], d_m33[:])

            # ---- branch accumulator tiles (persist across the plane loop)
            t_b = [bpool.tile([128, RG, CATW], dt.float16, tag=f"b{bi}", name=f"b{bi}") for bi in range(3)]
            # zero the W-pad columns (cols 0,1,130,131) once; products cover the rest
            for bi in range(3):
                nc.vector.memset(t_b[bi][:, :, 0:2], 0.0)
                nc.vector.memset(t_b[bi][:, :, 130:132], 0.0)

            # ---- two row-ranges; range-0 fuse chunks are interleaved into
            # range-1's plane loop so the PE stream never serializes a phase.
            RANGES = [(0, 18), (18, 34)]
            CATR0 = [0, 14]
            CATN = [18, 20]
            FCH = [range(0, 4), range(4, 8)]

            t_cat = [[[catpool.tile([128, CATN[ri], CATW], dt.float16,
                                    tag=f"cat{ri}{kt}{g}", name=f"cat{ri}{kt}{g}")
                       for g in range(2)] for kt in range(2)] for ri in range(2)]

            state = {"t_out": None, "gran": 0}

            def granules(R0, R1):
                out, c = [], R0
                while c < R1:
                    out.append((c, min(c + 8, R1)))
                    c = min(c + 8, R1)
                return out

            def emit_gen_plane(j, R0, R1):
                kpl = kpool.tile([128, R1 - R0, W], dt.float16, tag="kpl", name="kpl")
                for (c0, c1) in granules(R0, R1):
                    npx = (c1 - c0) * W
                    ps = genps.tile([128, 1024], dt.float32, tag="genps", name="genps")
                    for s0 in range(c0, c1, 4):
                        s1 = min(s0 + 4, c1)
                        nc.tensor.matmul(
                            ps[:, (s0 - c0) * W:(s1 - c0) * W],
                            t_gw[:, 128 * j:128 * j + 128],
                            t_ys[:, s0:s1, :],
                            start=True, stop=True,
                            tile_position=(0, 0),
                        )
                    dst = kpl[:, c0 - R0:c1 - R0, :]
                    state["gran"] += 1
                    if pool_evac and state["gran"] % pool_evac == 0:
                        nc.gpsimd.tensor_scalar_add(
                            dst.rearrange("p r w -> p (r w)"),
                            ps[:, 0:npx], t_gb[:, j:j + 1],
                        )
                    else:
                        nc.scalar.activation(
                            dst, ps[:, 0:npx].rearrange("p (r w) -> p r w", w=W),
                            IDENT, bias=t_gb[:, j:j + 1], scale=1.0,
                        )
                return kpl

            def emit_products(j, kpl, R0, R1):
                if "prod" in _skip:
                    nc.vector.tensor_copy(t_b[0][:, 0:1, 0:1], kpl[:, 0:1, 0:1])
                    return
                bi, t = j // 9, j % 9
                d = DIL[bi]
                dh, dw = t // 3 - 1, t % 3 - 1
                nrows = R1 - R0
                row_ofs = 5 + dh * d
                col_ofs = 5 + dw * d
                if col_ofs % 2 == 0:
                    xv = t_xe[:, row_ofs + R0:row_ofs + R1, col_ofs:col_ofs + W]
                else:
                    xv = t_xo[:, row_ofs + R0:row_ofs + R1, col_ofs - 1:col_ofs - 1 + W]
                bint = t_b[bi][:, R0:R1, 2:2 + W]
                if t == 0:
                    nc.vector.tensor_tensor(bint, kpl[:, 0:nrows, :], xv, MULT)
                else:
                    prod = prodpool.tile([128, 18, W], dt.float16, tag="prod", name="prod")
                    nc.vector.tensor_tensor(prod[:, 0:nrows, :], kpl[:, 0:nrows, :], xv, MULT)
                    nc.vector.tensor_tensor(bint, bint, prod[:, 0:nrows, :], ADD)

            def emit_fuse_chunk(ri, ch):
                if "fuse" in _skip:
                    return
                cr0 = CATR0[ri]
                if ch % 2 == 0:
                    state["t_out"] = outpool.tile([128, 8, W], dt.float32, tag="out", name="outt")
                t_out = state["t_out"]
                ps = fuseps.tile([128, 512], dt.float32, tag="fuseps", name="psA")
                ops = [(kt, s) for kt in range(2) for s in range(9)]
                for g in range(2):
                    for i, (kt, s) in enumerate(ops):
                        sh, sw = s // 3 - 1, s % 3 - 1
                        rr = 1 + 4 * ch + sh - cr0
                        blk = slice((kt * 9 + s) * 64, (kt * 9 + s) * 64 + 64)
                        nc.tensor.matmul(
                            ps[64 * g:64 * g + 64, :], t_fwT[0:128, blk],
                            t_cat[ri][kt][g][0:128, rr:rr + 4, 2 + sw:2 + sw + W],
                            start=(i == 0), stop=(i == len(ops) - 1),
                            tile_position=(0, 64 * g),
                        )
                oview = t_out[:, 4 * (ch % 2):4 * (ch % 2) + 4, :]
                nc.scalar.activation(
                    oview, ps[:].rearrange("p (r w) -> p r w", w=W),
                    IDENT, bias=t_fb[:, 0:1], scale=1.0,
                )
                if ch % 2 == 1:
                    nc.gpsimd.dma_start(d_out[:, 4 * ch - 4:4 * ch + 4, :], t_out[:])

            def emit_mask_and_cat(ri):
                mrow = 0 if ri == 0 else 33
                mt = t_m0 if ri == 0 else t_m33
                for bi in range(3):
                    nc.vector.tensor_scalar_mul(t_b[bi][:, mrow, :], t_b[bi][:, mrow, :], mt[:, 0:1])
                cr0, crn = CATR0[ri], CATN[ri]
                for g in range(2):
                    sl = slice(64 * g, 64 * g + 64)
                    nc.scalar.dma_start(t_cat[ri][0][g][0:64, :, :],
                                      t_xe[sl, 5 + cr0:5 + cr0 + crn, 3:3 + CATW])
                    nc.scalar.dma_start(t_cat[ri][0][g][64:128, :, :], t_b[0][sl, cr0:cr0 + crn, :])
                    nc.scalar.dma_start(t_cat[ri][1][g][0:64, :, :], t_b[1][sl, cr0:cr0 + crn, :])
                    nc.scalar.dma_start(t_cat[ri][1][g][64:128, :, :], t_b[2][sl, cr0:cr0 + crn, :])

            # range 0: gen + products
            for j in range(27):
                kpl = emit_gen_plane(j, 0, 18)
                emit_products(j, kpl, 0, 18)
            emit_mask_and_cat(0)

            # range 1 with range-0 fuse chunks interleaved (PE program order!)
            fuse0 = list(FCH[0])
            for j in range(27):
                kpl = emit_gen_plane(j, 18, 34)
                emit_products(j, kpl, 18, 34)
                if j in (5, 11, 17, 23):
                    emit_fuse_chunk(0, fuse0.pop(0))
            while fuse0:
                emit_fuse_chunk(0, fuse0.pop(0))
            emit_mask_and_cat(1)
            for ch in FCH[1]:
                emit_fuse_chunk(1, ch)

    nc.compile()
    return nc


# ----------------------------------------------------------------- runner
_CACHE = {}


def _get_runner(reps=1):
    """Build (once) a persistent jitted 8-core runner: fn(core_inputs) -> [out]*8."""
    key = ("runner", reps)
    if key in _CACHE:
        return _CACHE[key]

    import jax
    import numpy as _np
    from jax.sharding import Mesh, PartitionSpec
    from jax.experimental.shard_map import shard_map
    from concourse import mybir
    from concourse.bass2jax import install_neuronx_cc_hook, _bass_exec_p, partition_id_tensor

    nc = _build_nc(reps)
    install_neuronx_cc_hook()

    partition_name = nc.partition_id_tensor.name if nc.partition_id_tensor else None
    in_names, out_names, out_avals = [], [], []
    for alloc in nc.m.functions[0].allocations:
        if not isinstance(alloc, mybir.MemoryLocationSet):
            continue
        name = alloc.memorylocations[0].name
        if alloc.kind == "ExternalInput":
            if name != partition_name:
                in_names.append(name)
        elif alloc.kind == "ExternalOutput":
            out_names.append(name)
            out_avals.append(
                jax.core.ShapedArray(tuple(alloc.tensor_shape), mybir.dt.np(alloc.dtype))
            )
    n_params = len(in_names)
    n_outs = len(out_names)
    all_names = in_names + out_names
    if partition_name is not None:
        all_names = all_names + [partition_name]

    def _body(*args):
        operands = list(args)
        if partition_name is not None:
            operands.append(partition_id_tensor())
        outs = _bass_exec_p.bind(
            *operands,
            out_avals=tuple(out_avals),
            in_names=tuple(all_names),
            out_names=tuple(out_names),
            lowering_input_output_aliases=(),
            sim_require_finite=True,
            sim_require_nnan=True,
            nc=nc,
        )
        return tuple(outs)

    devices = jax.devices()[:NCORES]
    mesh = Mesh(_np.asarray(devices), ("core",))
    in_specs = (PartitionSpec("core"),) * (n_params + n_outs)
    out_specs = (PartitionSpec("core"),) * n_outs
    sharded = jax.jit(
        shard_map(_body, mesh=mesh, in_specs=in_specs, out_specs=out_specs, check_rep=False),
        keep_unused=True,
    )

    zero_shapes = [(NCORES * a.shape[0], *a.shape[1:]) for a in out_avals]
    zero_dtypes = [a.dtype for a in out_avals]

    def run(concat_inputs):
        """concat_inputs: list (len n_params, order in_names) of [NCORES*dim0, ...]."""
        zeros = [_np.zeros(s, d) for s, d in zip(zero_shapes, zero_dtypes)]
        out_arrs = sharded(*concat_inputs, *zeros)
        return [
            _np.asarray(out_arrs[i]).reshape(NCORES, *out_avals[i].shape)
            for i in range(n_outs)
        ]

    _CACHE[key] = (run, in_names, out_names)
    _CACHE[("raw", reps)] = dict(sharded=sharded, zero_shapes=zero_shapes, zero_dtypes=zero_dtypes,
                                 n_params=n_params, n_outs=n_outs, out_avals=out_avals, mesh=mesh)
    return _CACHE[key]


def make_concat_inputs(x, y, gen_w, gen_b, fuse_w, fuse_b, in_names):
    per_core = _prep_cores(np.asarray(x, F32), np.asarray(y, F32))
    wts = _prep_weights(
        np.asarray(gen_w, F32), np.asarray(gen_b, F32),
        np.asarray(fuse_w, F32), np.asarray(fuse_b, F32),
    )
    cat = []
    for name in in_names:
        if name in per_core:
            a = per_core[name]
            cat.append(a.reshape(NCORES * a.shape[1], *a.shape[2:]))
        else:
            w = wts[name]
            cat.append(np.concatenate([w] * NCORES, axis=0))
    return cat


def unpack_output(outs):
    """outs: [NCORES, 128, OUTR, W] fp32 -> [N, C, H, W]."""
    res = np.empty((N, C, H, W), F32)
    for core in range(NCORES):
        n, hh = core // 2, core % 2
        h0 = 64 * hh
        for g in range(2):
            res[n, :, h0 + 32 * g:h0 + 32 * g + 32, :] = outs[core, 64 * g:64 * g + 64]
    return res


def kernel(x, y, gen_w, gen_b, fuse_w, fuse_b):
    run, in_names, out_names = _get_runner()
    cat = make_concat_inputs(x, y, gen_w, gen_b, fuse_w, fuse_b, in_names)
    outs = run(cat)
    return unpack_output(outs[out_names.index("out")])
